# revision 1
# baseline (speedup 1.0000x reference)
"""DecoderRNN (bidirectional-GRU greedy decoder) Trainium2 kernel, 8-core SPMD.

Strategy:
  - Vocab-parallel: each core owns a 4000-row slice of w_out / b_out and
    computes its logits slice each step.
  - GRU tensor-parallel: each core computes a 128-wide slice of each gate
    (both directions); hidden state is AllGathered (transposed layout) each
    step so every core has the full h for the next step's matmuls and for
    the output projection.
  - Greedy argmax: per-core top-1 (value, index) via vector.max/max_index,
    AllGather of the 8 candidates, local combine -> next token; embedding
    row gathered from a replicated table via indirect DMA.
  - log_softmax: per-core sum(exp(logits - m_glob)) via ACT accum_out,
    AllGather of partial sums, logZ = m + ln(S); logp written per step.
  - w_out slice (2048 x 4000 fp32, transposed) is too big for SBUF: 1500
    columns stay resident, 2500 columns are re-streamed from HBM each step.

Layouts (per core k, v0 = 4000*k, hidden slice = 128*k):
  wres   [128, 16*1500]  resident w_outT: [p, c*1500+j] = w_out[v0+j, c*128+p]
  wstream[16*5*128, 500] streamed tiles in (K-chunk, s) order
  wih/whh[128, 8*768]    gate-sliced GRU weights, transposed; column order
                         per K-chunk: [f_r f_z b_r b_z | f_n b_n] (128 each)
  hT     [128, 8*64]     full hidden transposed: [p, c*64 + dir*32 + b]
  xT     [128, 8*32]     embedded token transposed: [p, c*32 + b]
  logits [128, 1000]     [32*j + b, g*500 + f] = logits[b, v0+(g*4+j)*500+f]
"""

import numpy as np

import concourse.bass as bass
import concourse.bacc as bacc
import concourse.mybir as mybir
import concourse.tile as tile
import concourse.bass_utils as bass_utils
from concourse.masks import make_identity

F32 = mybir.dt.float32
U32 = mybir.dt.uint32
AF = mybir.ActivationFunctionType
ALU = mybir.AluOpType
AX = mybir.AxisListType

B = 32
H = 1024
V = 32000
NC = 8
Vs = V // NC          # 4000 vocab rows per core
Hs = H // NC          # 128 hidden dims per core
KC = 16               # K-chunks of 128 over 2H
NCH = 8               # n-chunks of 500 over Vs
CH = 500              # n-chunk width (one PSUM bank)
RES = 3               # default resident n-chunks
STR = NCH - RES       # (per-build values passed explicitly)
GROUPS = 2            # col-tile groups of 4 chunks
BIG = 1.0e30


def build_program(T: int, debug: bool = False, res: int = RES,
                  col_tile: bool = True, fill: int = 0,
                  no_proj: bool = False, fake_stream: bool = False,
                  no_ag13: bool = False, no_ag1: bool = False,
                  no_ag2: bool = False):
    STR = NCH - res
    RES = res
    nc = bacc.Bacc("TRN2", target_bir_lowering=False, debug=False, num_devices=NC)
    dbg = {}
    if debug:
        dbg["srz"] = nc.dram_tensor("dbg_srz", [B, 512], F32, kind="ExternalOutput")
        dbg["n"] = nc.dram_tensor("dbg_n", [B, 256], F32, kind="ExternalOutput")
        dbg["hnew"] = nc.dram_tensor("dbg_hnew", [B, 256], F32, kind="ExternalOutput")
        dbg["logits"] = nc.dram_tensor("dbg_logits", [128, 1000], F32, kind="ExternalOutput")
        dbg["mg"] = nc.dram_tensor("dbg_mg", [B, 1], F32, kind="ExternalOutput")
        dbg["ig"] = nc.dram_tensor("dbg_ig", [B, 1], F32, kind="ExternalOutput")
        dbg["sg"] = nc.dram_tensor("dbg_sg", [B, 1], F32, kind="ExternalOutput")
        dbg["hT"] = nc.dram_tensor("dbg_hT", [128, 512], F32, kind="ExternalOutput")
        dbg["rzps"] = nc.dram_tensor("dbg_rzps", [B, 512], F32, kind="ExternalOutput")
        dbg["inhnps"] = nc.dram_tensor("dbg_inhnps", [B, 512], F32, kind="ExternalOutput")
        dbg["srz_pre"] = nc.dram_tensor("dbg_srz_pre", [B, 512], F32, kind="ExternalOutput")

    emb_t = nc.dram_tensor("emb_t", [V, H], F32, kind="ExternalInput")
    wres_t = nc.dram_tensor("wres_t", [128, KC * RES * CH], F32, kind="ExternalInput")
    wstream_t = nc.dram_tensor("wstream_t", [max(KC * STR, 1) * 128, CH], F32, kind="ExternalInput")
    wih_t = nc.dram_tensor("wih_t", [128, 8 * 768], F32, kind="ExternalInput")
    whh_t = nc.dram_tensor("whh_t", [128, 8 * 768], F32, kind="ExternalInput")
    brz_t = nc.dram_tensor("brz_t", [B, 512], F32, kind="ExternalInput")
    bin_t = nc.dram_tensor("bin_t", [B, 256], F32, kind="ExternalInput")
    bhn_t = nc.dram_tensor("bhn_t", [B, 256], F32, kind="ExternalInput")
    bout_t = nc.dram_tensor("bout_t", [128, GROUPS * CH], F32, kind="ExternalInput")
    offs_t = nc.dram_tensor("offs_t", [128, 1], F32, kind="ExternalInput")
    ht0_t = nc.dram_tensor("ht0_t", [128, 8 * 64], F32, kind="ExternalInput")
    hbm0_t = nc.dram_tensor("hbm0_t", [B, 256], F32, kind="ExternalInput")
    x0t_t = nc.dram_tensor("x0t_t", [128, 8 * 32], F32, kind="ExternalInput")
    logp_t = nc.dram_tensor("logp_t", [T * 128, GROUPS * CH], F32, kind="ExternalOutput")

    rg = [list(range(NC))]

    with tile.TileContext(nc) as tc:
        with (
            tc.tile_pool(name="const", bufs=1) as cpool,
            tc.tile_pool(name="stream", bufs=3) as spool,
            tc.tile_pool(name="gate", bufs=1) as gpool,
            tc.tile_pool(name="lg", bufs=2) as lpool,
            tc.tile_pool(name="stats", bufs=2) as tpool,
            tc.tile_pool(name="ps_rz", bufs=1, space="PSUM") as ps_rz_pool,
            tc.tile_pool(name="ps_n", bufs=1, space="PSUM") as ps_n_pool,
            tc.tile_pool(name="ps_proj", bufs=1, space="PSUM") as ps_proj_pool,
            tc.tile_pool(name="ps_tr", bufs=1, space="PSUM") as ps_tr_pool,
            tc.tile_pool(name="ps_exp", bufs=1, space="PSUM") as ps_exp_pool,
            tc.tile_pool(name="dram", bufs=2, space="DRAM") as dpool,
        ):
            # ---- resident loads ----
            ident = cpool.tile([128, 128], F32, name="ident")
            make_identity(nc, ident[:])
            id32 = ident[0:32, 0:32]
            wres = cpool.tile([128, KC * RES * CH], F32, name="wres")
            nc.sync.dma_start(wres[:], wres_t.ap())
            wih = cpool.tile([128, 8 * 768], F32, name="wih")
            nc.sync.dma_start(wih[:], wih_t.ap())
            whh = cpool.tile([128, 8 * 768], F32, name="whh")
            nc.sync.dma_start(whh[:], whh_t.ap())
            brz = cpool.tile([B, 512], F32, name="brz")
            nc.sync.dma_start(brz[:], brz_t.ap())
            b_in = cpool.tile([B, 256], F32, name="b_in")
            nc.sync.dma_start(b_in[:], bin_t.ap())
            b_hn = cpool.tile([B, 256], F32, name="b_hn")
            nc.sync.dma_start(b_hn[:], bhn_t.ap())
            bout = cpool.tile([128, GROUPS * CH], F32, name="bout")
            nc.sync.dma_start(bout[:], bout_t.ap())
            offs = cpool.tile([128, 1], F32, name="offs")
            nc.sync.dma_start(offs[:], offs_t.ap())
            bigt = cpool.tile([B, 8], F32, name="bigt")
            nc.vector.memset(bigt[:], BIG)

            # ping-pong state
            hT = [cpool.tile([128, 8 * 64], F32, name=f"hT{i}") for i in range(2)]
            xT = [cpool.tile([128, 8 * 32], F32, name=f"xT{i}") for i in range(2)]
            hbm = [cpool.tile([B, 256], F32, name=f"hbm{i}") for i in range(2)]
            nc.sync.dma_start(hT[0][:], ht0_t.ap())
            nc.sync.dma_start(xT[0][:], x0t_t.ap())
            nc.sync.dma_start(hbm[0][:], hbm0_t.ap())

            def emit_gh(t, rz_ps, hn_ps):
                """h-side GRU matmuls for step t (reads hT[t%2] = h(t-1))."""
                h = hT[t % 2]
                for c in range(8):
                    hf = h[:, c * 64 : c * 64 + 32]
                    hb = h[:, c * 64 + 32 : c * 64 + 64]
                    w = whh[:, c * 768 : (c + 1) * 768]
                    # start=True zeroes the whole 2KB PSUM bank: exactly one
                    # bank-clearing MM per bank per step, everything else adds.
                    nc.tensor.matmul(rz_ps[:, 0:256], lhsT=hf, rhs=w[:, 0:256],
                                     start=(c == 0), stop=False)
                    nc.tensor.matmul(rz_ps[:, 256:512], lhsT=hb, rhs=w[:, 256:512],
                                     start=False, stop=False)
                    nc.tensor.matmul(hn_ps[:, 0:128], lhsT=hf, rhs=w[:, 512:640],
                                     start=(c == 0), stop=False)
                    nc.tensor.matmul(hn_ps[:, 128:256], lhsT=hb, rhs=w[:, 640:768],
                                     start=False, stop=False)

            # step-0 h-side prologue
            rz_ps_next = ps_rz_pool.tile([B, 512], F32, name="rz_ps", tag="rz")
            inhn_ps_next = ps_n_pool.tile([B, 512], F32, name="inhn_ps", tag="inhn")
            emit_gh(0, rz_ps_next, inhn_ps_next[:, 0:256])

            for t in range(T):
                rz_ps = rz_ps_next
                inhn_ps = inhn_ps_next
                hn_ps = inhn_ps[:, 0:256]
                in_ps = inhn_ps[:, 256:512]
                x = xT[t % 2]
                h_prev = hbm[t % 2]
                h_cur = hT[(t + 1) % 2]   # written by AG1(t)

                # ---- x-side GRU matmuls ----
                for c in range(8):
                    xc = x[:, c * 32 : (c + 1) * 32]
                    w = wih[:, c * 768 : (c + 1) * 768]
                    nc.tensor.matmul(rz_ps[:], lhsT=xc, rhs=w[:, 0:512],
                                     start=False, stop=(c == 7))
                    nc.tensor.matmul(in_ps, lhsT=xc, rhs=w[:, 512:768],
                                     start=False, stop=(c == 7))

                # ---- gates (batch-major; col order [f_r f_z b_r b_z]) ----
                if debug and t == 0:
                    rzc = tpool.tile([B, 512], F32, name="rzc", tag="rzc")
                    nc.vector.tensor_copy(rzc[:], rz_ps[:])
                    nc.sync.dma_start(dbg["rzps"].ap(), rzc[:])
                    ihc = tpool.tile([B, 512], F32, name="ihc", tag="ihc")
                    nc.vector.tensor_copy(ihc[:], inhn_ps[:])
                    nc.sync.dma_start(dbg["inhnps"].ap(), ihc[:])
                s_rz = gpool.tile([B, 512], F32, name="s_rz", tag="s_rz")
                nc.vector.tensor_add(s_rz[:], rz_ps[:], brz[:])
                if debug and t == 0:
                    nc.sync.dma_start(dbg["srz_pre"].ap(), s_rz[:])
                nc.scalar.activation(s_rz[:], s_rz[:], AF.Tanh, scale=0.5)
                nc.vector.tensor_scalar(s_rz[:], s_rz[:], 0.5, 0.5,
                                        op0=ALU.mult, op1=ALU.add)
                if debug and t == 0:
                    nc.sync.dma_start(dbg["srz"].ap(), s_rz[:])
                i_n = gpool.tile([B, 256], F32, name="i_n", tag="i_n")
                nc.vector.tensor_add(i_n[:], in_ps, b_in[:])
                h_n = gpool.tile([B, 256], F32, name="h_n", tag="h_n")
                nc.vector.tensor_add(h_n[:], hn_ps, b_hn[:])
                # h_n *= r ; h_n += i_n ; n = tanh(h_n)
                nc.vector.tensor_tensor(h_n[:, 0:128], s_rz[:, 0:128],
                                        h_n[:, 0:128], op=ALU.mult)
                nc.vector.tensor_tensor(h_n[:, 128:256], s_rz[:, 256:384],
                                        h_n[:, 128:256], op=ALU.mult)
                nc.vector.tensor_add(h_n[:], h_n[:], i_n[:])
                nc.scalar.activation(h_n[:], h_n[:], AF.Tanh)
                if debug and t == 0:
                    nc.sync.dma_start(dbg["n"].ap(), h_n[:])
                # d = (h_prev - n) * z ; h_new = n + d   (d reuses i_n)
                nc.vector.tensor_sub(i_n[:], h_prev[:], h_n[:])
                nc.vector.tensor_tensor(i_n[:, 0:128], s_rz[:, 128:256],
                                        i_n[:, 0:128], op=ALU.mult)
                nc.vector.tensor_tensor(i_n[:, 128:256], s_rz[:, 384:512],
                                        i_n[:, 128:256], op=ALU.mult)
                h_new = hbm[(t + 1) % 2]
                nc.vector.tensor_add(h_new[:], h_n[:], i_n[:])
                if debug and t == 0:
                    nc.sync.dma_start(dbg["hnew"].ap(), h_new[:])

                # ---- transpose h_new, AllGather hidden ----
                tr_ps = ps_tr_pool.tile([128, 512], F32, name="tr_ps", tag="tr")
                nc.tensor.matmul(tr_ps[:, 0:32], lhsT=h_new[:, 0:128], rhs=id32,
                                 is_transpose=True, start=True, stop=False)
                nc.tensor.matmul(tr_ps[:, 32:64], lhsT=h_new[:, 128:256], rhs=id32,
                                 is_transpose=True, start=False, stop=True)
                ag1_sb = tpool.tile([128, 64], F32, name="ag1_sb", tag="ag1_sb")
                nc.vector.tensor_copy(ag1_sb[:], tr_ps[:, 0:64])
                ag1_in = dpool.tile([128, 64], F32, name="ag1_in", tag="ag1_in")
                nc.gpsimd.dma_start(ag1_in[:], ag1_sb[:])
                ag1_out = dpool.tile([128 * NC, 64], F32, name="ag1_out",
                                     addr_space="Shared", tag="ag1_out")
                if not (no_ag13 or no_ag1):
                    nc.gpsimd.collective_compute(
                        "AllGather", ALU.bypass, replica_groups=rg,
                        ins=[ag1_in.opt()], outs=[ag1_out.opt()])
                elif no_ag1:
                    nc.gpsimd.dma_start(
                        ag1_out[:].rearrange("(c p) q -> p c q", p=128),
                        ag1_in[:].rearrange("p (c q) -> p c q", c=1).to_broadcast([128, 8, 64]))
                else:
                    nc.sync.dma_start(
                        ag1_out[:].rearrange("(c p) q -> c p q", p=128)[0:1],
                        ag1_in[:].rearrange("(c p) q -> c p q", c=1))
                nc.gpsimd.dma_start(
                    h_cur[:].rearrange("p (c q) -> p c q", c=8),
                    ag1_out[:].rearrange("(c p) q -> p c q", p=128))
                if debug and t == 0:
                    nc.sync.dma_start(dbg["hT"].ap(), h_cur[:])

                # ---- output projection ----
                pj = [ps_proj_pool.tile([128, 512], F32, name=f"pj{g}", tag=f"pj{g}")
                      for g in range(GROUPS)]
                def lh_of(c):
                    if c < 8:
                        return h_cur[:, c * 64 : c * 64 + 32]
                    return h_cur[:, (c - 8) * 64 + 32 : (c - 8) * 64 + 64]

                def proj_mm(c, ch, rhs):
                    g, j = divmod(ch, 4)
                    kw = {"tile_position": (0, 32 * j)} if col_tile else {}
                    nc.tensor.matmul(
                        pj[g][32 * j : 32 * (j + 1), 0:CH], lhsT=lh_of(c),
                        rhs=rhs, start=(c == 0), stop=(c == KC - 1),
                        skip_group_check=True, **kw)

                sts = []
                if not no_proj:
                    # streamed-tile DMAs first (maximize prefetch window), then
                    # all resident MMs (dense burst warms PE), then streamed MMs
                    for c in range(KC):
                        if STR and not fake_stream:
                            st = spool.tile([128, STR * CH], F32, name="st", tag="st")
                            for s in range(STR):
                                nc.sync.dma_start(
                                    st[:, s * CH : (s + 1) * CH],
                                    wstream_t.ap()[(c * STR + s) * 128 : (c * STR + s + 1) * 128, :])
                            sts.append(st)
                        for ch in range(RES):
                            proj_mm(c, ch, wres[:, (c * RES + ch) * CH : (c * RES + ch + 1) * CH])
                    for c in range(KC):
                        for ch in range(RES, NCH):
                            if fake_stream:
                                rhs = wres[:, (c * RES + RES - 1) * CH : (c * RES + RES) * CH]
                            else:
                                rhs = sts[c][:, (ch - RES) * CH : (ch - RES + 1) * CH]
                            proj_mm(c, ch, rhs)

                if fill:
                    fill_ps = ps_exp_pool.tile([128, 512], F32, name="fill_ps", tag="exp")
                    for fi in range(fill):
                        nc.tensor.matmul(fill_ps[0:32, 0:CH],
                                         lhsT=wres[:, 0:32], rhs=wres[:, 0:CH],
                                         start=True, stop=True,
                                         skip_group_check=True)
                # ---- logits epilogue: bias, max, argmax ----
                logits = lpool.tile([128, GROUPS * CH], F32, name="logits", tag="logits")
                cand = tpool.tile([B, 4], F32, name="cand", tag="cand")
                candi = tpool.tile([B, 4], F32, name="candi", tag="candi")
                for g in range(GROUPS):
                    lg = logits[:, g * CH : (g + 1) * CH]
                    if no_proj:
                        nc.vector.tensor_copy(lg, bout[:, g * CH : (g + 1) * CH])
                    else:
                        nc.vector.tensor_add(lg, pj[g][:, 0:CH], bout[:, g * CH : (g + 1) * CH])
                if debug and t == 0:
                    nc.sync.dma_start(dbg["logits"].ap(), logits[:])
                mx8 = tpool.tile([128, 8], F32, name="mx8", tag="mx8")
                ix8 = tpool.tile([128, 8], U32, name="ix8", tag="ix8")
                nc.vector.max(out=mx8[:], in_=logits[:])
                nc.vector.max_index(out=ix8[:], in_max=mx8[:], in_values=logits[:])
                # vocab index = offs(j) + idx + (idx >= 500) * 1500
                ixf = tpool.tile([128, 1], F32, name="ixf", tag="ixf")
                nc.vector.tensor_copy(ixf[:], ix8[:, 0:1])
                gmask = tpool.tile([128, 1], F32, name="gmask", tag="gmask")
                nc.vector.tensor_scalar(gmask[:], ixf[:], float(CH), 1500.0,
                                        op0=ALU.is_ge, op1=ALU.mult)
                nc.vector.tensor_add(ixf[:], ixf[:], gmask[:])
                nc.vector.tensor_add(ixf[:], ixf[:], offs[:])
                for j in range(4):
                    nc.vector.tensor_copy(cand[:, j : j + 1],
                                          mx8[32 * j : 32 * (j + 1), 0:1])
                    nc.vector.tensor_copy(candi[:, j : j + 1],
                                          ixf[32 * j : 32 * (j + 1), 0:1])
                m_loc = tpool.tile([B, 1], F32, name="m_loc", tag="m_loc")
                nc.vector.reduce_max(m_loc[:], cand[:], axis=AX.X)
                msk = tpool.tile([B, 4], U32, name="msk", tag="msk")
                nc.vector.tensor_scalar(msk[:], cand[:], m_loc[:], None, op0=ALU.is_equal)
                isel = tpool.tile([B, 4], F32, name="isel", tag="isel")
                nc.vector.tensor_copy(isel[:], bigt[:, 0:4])
                nc.vector.copy_predicated(isel[:], msk[:], candi[:])
                i_loc = tpool.tile([B, 1], F32, name="i_loc", tag="i_loc")
                nc.vector.tensor_reduce(i_loc[:], isel[:], axis=AX.X, op=ALU.min)

                # ---- local sum-exp (vs local max) before AG2 ----
                mneg_l = tpool.tile([128, 1], F32, name="mneg_l", tag="mneg_l")
                nc.vector.tensor_scalar_mul(mneg_l[0:B, :], m_loc[:], -1.0)
                nc.vector.tensor_copy(mneg_l[B : 2 * B, :], mneg_l[0:B, :])
                nc.vector.tensor_copy(mneg_l[2 * B :, :], mneg_l[0 : 2 * B, :])
                sparts = tpool.tile([128, 2], F32, name="sparts", tag="sparts")
                for g in range(GROUPS):
                    e_ps = ps_exp_pool.tile([128, 512], F32, name="e_ps", tag="exp")
                    nc.scalar.activation(e_ps[:, 0:CH], logits[:, g * CH : (g + 1) * CH],
                                         AF.Exp, bias=mneg_l[:, 0:1],
                                         accum_out=sparts[:, g : g + 1])
                s128 = tpool.tile([128, 1], F32, name="s128", tag="s128")
                nc.vector.tensor_add(s128[:], sparts[:, 0:1], sparts[:, 1:2])
                scand = tpool.tile([B, 4], F32, name="scand", tag="scand")
                for j in range(4):
                    nc.vector.tensor_copy(scand[:, j : j + 1],
                                          s128[32 * j : 32 * (j + 1), :])
                s_loc = tpool.tile([B, 1], F32, name="s_loc", tag="s_loc")
                nc.vector.reduce_sum(s_loc[:], scand[:], axis=AX.X)

                # ---- AG2: (m, idx, s) from all cores; global argmax + logZ ----
                ag2_sb = tpool.tile([B, 3], F32, name="ag2_sb", tag="ag2_sb")
                nc.vector.tensor_copy(ag2_sb[:, 0:1], m_loc[:])
                nc.vector.tensor_copy(ag2_sb[:, 1:2], i_loc[:])
                nc.vector.tensor_copy(ag2_sb[:, 2:3], s_loc[:])
                ag2_in = dpool.tile([B, 3], F32, name="ag2_in", tag="ag2_in")
                nc.gpsimd.dma_start(ag2_in[:], ag2_sb[:])
                ag2_out = dpool.tile([B * NC, 3], F32, name="ag2_out",
                                     addr_space="Shared", tag="ag2_out")
                if not no_ag2:
                    nc.gpsimd.collective_compute(
                        "AllGather", ALU.bypass, replica_groups=rg,
                        ins=[ag2_in.opt()], outs=[ag2_out.opt()])
                else:
                    nc.gpsimd.dma_start(
                        ag2_out[:].rearrange("(r b) c -> r b c", b=B),
                        ag2_in[:].rearrange("(r b) c -> r b c", r=1).to_broadcast([NC, B, 3]))
                unp2 = tpool.tile([B, 24], F32, name="unp2", tag="unp2")
                nc.gpsimd.dma_start(
                    unp2[:].rearrange("b (r c) -> b r c", r=NC),
                    ag2_out[:].rearrange("(r b) c -> b r c", b=B))
                vals = bass.AP(unp2.tensor, unp2[:].offset,
                               [unp2[:].ap[0], [3, 8]])
                idxs = bass.AP(unp2.tensor, unp2[:].offset + 1,
                               [unp2[:].ap[0], [3, 8]])
                svals = bass.AP(unp2.tensor, unp2[:].offset + 2,
                                [unp2[:].ap[0], [3, 8]])
                m_glob = tpool.tile([B, 1], F32, name="m_glob", tag="m_glob")
                nc.vector.reduce_max(m_glob[:], vals, axis=AX.X)
                msk2 = tpool.tile([B, 8], U32, name="msk2", tag="msk2")
                nc.vector.tensor_scalar(msk2[:], vals, m_glob[:], None, op0=ALU.is_equal)
                isel2 = tpool.tile([B, 8], F32, name="isel2", tag="isel2")
                nc.vector.tensor_copy(isel2[:], bigt[:])
                nc.vector.copy_predicated(isel2[:], msk2[:], idxs)
                i_glob = tpool.tile([B, 1], F32, name="i_glob", tag="i_glob")
                nc.vector.tensor_reduce(i_glob[:], isel2[:], axis=AX.X, op=ALU.min)
                if debug and t == 0:
                    nc.sync.dma_start(dbg["mg"].ap(), m_glob[:])
                    nc.sync.dma_start(dbg["ig"].ap(), i_glob[:])
                # S_glob = sum_k s_k * exp(m_k - M); logZ = M + ln(S_glob)
                dmx = tpool.tile([B, 8], F32, name="dmx", tag="dmx")
                nc.vector.tensor_scalar(dmx[:], vals, m_glob[:], None, op0=ALU.subtract)
                nc.scalar.activation(dmx[:], dmx[:], AF.Exp)
                nc.vector.tensor_tensor(dmx[:], dmx[:], svals, op=ALU.mult)
                s_glob = tpool.tile([B, 1], F32, name="s_glob", tag="s_glob")
                nc.vector.reduce_sum(s_glob[:], dmx[:], axis=AX.X)
                if debug and t == 0:
                    nc.sync.dma_start(dbg["sg"].ap(), s_glob[:])
                lns = tpool.tile([B, 1], F32, name="lns", tag="lns")
                nc.scalar.activation(lns[:], s_glob[:], AF.Ln)
                logz = tpool.tile([128, 1], F32, name="logz", tag="logz")
                nc.vector.tensor_add(logz[0:B, :], lns[:], m_glob[:])
                nc.vector.tensor_copy(logz[B : 2 * B, :], logz[0:B, :])
                nc.vector.tensor_copy(logz[2 * B :, :], logz[0 : 2 * B, :])

                # ---- prefetch for t+1: gh matmuls, token embed, transpose ----
                if t + 1 < T:
                    rz_ps_next = ps_rz_pool.tile([B, 512], F32, name="rz_ps", tag="rz")
                    inhn_ps_next = ps_n_pool.tile([B, 512], F32, name="inhn_ps", tag="inhn")
                    emit_gh(t + 1, rz_ps_next, inhn_ps_next[:, 0:256])
                    tok = tpool.tile([B, 1], U32, name="tok", tag="tok")
                    nc.vector.tensor_copy(tok[:], i_glob[:])
                    x_sb = tpool.tile([B, H], F32, name="x_sb", tag="x_sb", bufs=1)
                    nc.gpsimd.indirect_dma_start(
                        out=x_sb[:], out_offset=None, in_=emb_t.ap(),
                        in_offset=bass.IndirectOffsetOnAxis(ap=tok[:, 0:1], axis=0))
                    xtr_ps = ps_tr_pool.tile([128, 512], F32, name="xtr_ps", tag="tr")
                    for c in range(8):
                        nc.tensor.matmul(xtr_ps[:, c * 32 : (c + 1) * 32],
                                         lhsT=x_sb[:, c * 128 : (c + 1) * 128],
                                         rhs=id32, is_transpose=True,
                                         start=(c == 0), stop=(c == 7))
                    nc.vector.tensor_copy(xT[(t + 1) % 2][:], xtr_ps[:, 0:256])

                # ---- logp = logits - logZ; write out ----
                nc.gpsimd.tensor_scalar(logits[:], logits[:], logz[:, 0:1], None,
                                        op0=ALU.subtract)
                nc.gpsimd.dma_start(logp_t.ap()[t * 128 : (t + 1) * 128, :], logits[:])

    nc.compile()
    return nc


def prep_inputs(inputs, hidden, emb, w_ih_f, w_hh_f, b_ih_f, b_hh_f,
                w_ih_b, w_hh_b, b_ih_b, b_hh_b, w_out, b_out):
    """Build the per-core input maps (all numpy, host-side sharding)."""
    emb = np.ascontiguousarray(np.asarray(emb), dtype=np.float32)
    w_out = np.asarray(w_out)
    tok0 = np.asarray(inputs)[:, 0].astype(np.int64)
    x0 = emb[tok0]                                              # (B, H)
    hidden = np.asarray(hidden)
    h_f0, h_b0 = hidden[0], hidden[1]                           # (B, H)

    x0t = np.ascontiguousarray(x0.T).reshape(8, 128, B).transpose(1, 0, 2) \
        .reshape(128, 8 * B).astype(np.float32)
    ht0 = np.empty((128, 8, 64), dtype=np.float32)
    ht0[:, :, 0:32] = np.ascontiguousarray(h_f0.T).reshape(8, 128, B).transpose(1, 0, 2)
    ht0[:, :, 32:64] = np.ascontiguousarray(h_b0.T).reshape(8, 128, B).transpose(1, 0, 2)
    ht0 = ht0.reshape(128, 8 * 64)

    wihf, whhf = np.asarray(w_ih_f), np.asarray(w_hh_f)
    wihb, whhb = np.asarray(w_ih_b), np.asarray(w_hh_b)
    bihf, bhhf = np.asarray(b_ih_f), np.asarray(b_hh_f)
    bihb, bhhb = np.asarray(b_ih_b), np.asarray(b_hh_b)

    in_maps = []
    for k in range(NC):
        v0 = Vs * k
        sl = [slice(g * H + Hs * k, g * H + Hs * (k + 1)) for g in range(3)]

        w_oT = np.ascontiguousarray(w_out[v0 : v0 + Vs, :].T)   # (2048, Vs)
        wres = w_oT.reshape(KC, 128, Vs)[:, :, : RES * CH] \
            .transpose(1, 0, 2).reshape(128, KC * RES * CH).astype(np.float32).copy()
        wstr = w_oT.reshape(KC, 128, NCH, CH)[:, :, RES:, :] \
            .transpose(0, 2, 1, 3).reshape(KC * STR * 128, CH).astype(np.float32).copy()

        def gcat(wf, wb):
            cols = [wf[sl[0]].T, wf[sl[1]].T, wb[sl[0]].T, wb[sl[1]].T,
                    wf[sl[2]].T, wb[sl[2]].T]
            cat = np.concatenate(cols, axis=1)                   # (1024, 768)
            return cat.reshape(8, 128, 768).transpose(1, 0, 2) \
                .reshape(128, 8 * 768).astype(np.float32).copy()

        def bcast(v):
            return np.broadcast_to(v.astype(np.float32), (B, v.size)).copy()

        brz = bcast(np.concatenate([bihf[sl[0]] + bhhf[sl[0]],
                                    bihf[sl[1]] + bhhf[sl[1]],
                                    bihb[sl[0]] + bhhb[sl[0]],
                                    bihb[sl[1]] + bhhb[sl[1]]]))
        b_in_ = bcast(np.concatenate([bihf[sl[2]], bihb[sl[2]]]))
        b_hn_ = bcast(np.concatenate([bhhf[sl[2]], bhhb[sl[2]]]))

        bo = np.asarray(b_out)[v0 : v0 + Vs].reshape(GROUPS, 4, CH)
        boutt = np.empty((128, GROUPS * CH), dtype=np.float32)
        for g in range(GROUPS):
            for j in range(4):
                boutt[32 * j : 32 * (j + 1), g * CH : (g + 1) * CH] = bo[g, j]

        # per-partition (32j+b) vocab base: v0 + j*500
        of = np.empty((128, 1), dtype=np.float32)
        for j in range(4):
            of[32 * j : 32 * (j + 1), 0] = v0 + j * CH

        hbm0 = np.concatenate([h_f0[:, Hs * k : Hs * (k + 1)],
                               h_b0[:, Hs * k : Hs * (k + 1)]], axis=1) \
            .astype(np.float32).copy()

        in_maps.append({
            "emb_t": emb, "wres_t": wres, "wstream_t": wstr,
            "wih_t": gcat(wihf, wihb), "whh_t": gcat(whhf, whhb),
            "brz_t": brz, "bin_t": b_in_, "bhn_t": b_hn_,
            "bout_t": boutt, "offs_t": of,
            "ht0_t": ht0, "hbm0_t": hbm0, "x0t_t": x0t,
        })
    return in_maps


_CACHE = {}


def _get_program(T, **kw):
    key = (T, tuple(sorted(kw.items())))
    if key not in _CACHE:
        _CACHE[key] = build_program(T, **kw)
    return _CACHE[key]


def run(T, in_maps, trace=False):
    nc = _get_program(T)
    res = bass_utils.run_bass_kernel_spmd(
        nc, in_maps, core_ids=list(range(NC)), trace=trace)
    outs = []
    for k in range(NC):
        arr = res.results[k]["logp_t"].reshape(T, 4, B, GROUPS, CH)
        outs.append(arr.transpose(2, 0, 3, 1, 4).reshape(B, T, Vs))
    return np.concatenate(outs, axis=2), res


def kernel(inputs, hidden, emb, w_ih_f, w_hh_f, b_ih_f, b_hh_f,
           w_ih_b, w_hh_b, b_ih_b, b_hh_b, w_out, b_out, output_len):
    T = int(output_len)
    in_maps = prep_inputs(inputs, hidden, emb, w_ih_f, w_hh_f, b_ih_f, b_hh_f,
                          w_ih_b, w_hh_b, b_ih_b, b_hh_b, w_out, b_out)
    out, _ = run(T, in_maps)
    return out



# revision 2
# speedup vs baseline: 1.0160x; 1.0160x over previous
"""DecoderRNN (bidirectional-GRU greedy decoder) Trainium2 kernel, 8-core SPMD.

v2 architecture ("resident fp16 + exact refinement"):
  - w_out sharded over vocab (4000 rows/core), stored RESIDENT in SBUF as
    fp16 (125KB/partition) -> projection is a single fp16 pass on PE
    (1 cyc/row vs fp32's 4), error <= ~7e-4, fine for the logp output.
  - Exact greedy argmax via refinement: per-partition top-2 of the fp16
    logits (vector.max top-8 instruction), narrowed to per-core top-2
    candidates; their w_out rows are gathered in fp32 (indirect DMA),
    transposed on PE, and exact fp32 logits recomputed; cross-core compare
    of EXACT values via AllGather -> token matches fp32 reference.
  - x-side GRU matmuls eliminated: E' = emb @ w_ih.T + biases precomputed
    on host (per-core gate slices, [V, 768] fp32); per step one indirect
    DMA gather of E'[token] replaces embed+transpose+8 matmuls.
  - h-side GRU matmuls stay fp32 sharded (128 gate-dims/core) with the
    hidden state AllGathered (transposed layout) each step.
  - log_softmax: per-core sum(exp(logits_hi - m_loc)) via ACT accum_out,
    logZ combined from AG2 stats (m_ex, i_ex, m_loc, s_loc).

Layouts (per core k, v0 = 4000*k, hidden slice = 128*k):
  whi  [128, 16*8*500] fp16: [p, (c*8+ch)*500+f] = w_out[v0+ch*500+f, c*128+p]
  whh  [128, 8*768]  gate-sliced GRU weights, transposed; column order
                     per K-chunk: [f_r f_z b_r b_z | f_n b_n] (128 each)
  hT   [128, 8*64]   full hidden transposed: [p, c*64 + dir*32 + b]
  eg_t [V, 768]      E' rows: [f_r f_z b_r b_z | f_n b_n] slices of core k
  wb_t [4000, 2052]  w_out rows (fp32) + b_out + 3 pad, for refinement
  logits [128, 1000] [32*j + b, g*500 + f] = logits[b, v0+(g*4+j)*500+f]
"""

import numpy as np

import concourse.bass as bass
import concourse.bacc as bacc
import concourse.mybir as mybir
import concourse.tile as tile
import concourse.bass_utils as bass_utils
from concourse.masks import make_identity

F32 = mybir.dt.float32
F16 = mybir.dt.float16
U32 = mybir.dt.uint32
AF = mybir.ActivationFunctionType
ALU = mybir.AluOpType
AX = mybir.AxisListType

B = 32
H = 1024
V = 32000
NC = 8
Vs = V // NC          # 4000 vocab rows per core
Hs = H // NC          # 128 hidden dims per core
KC = 16               # K-chunks of 128 over 2H
NCH = 8               # n-chunks of 500 over Vs
CH = 500              # n-chunk width (one PSUM bank)
GROUPS = 2            # logits col groups of 500
WBC = 2052            # refinement row width (2048 w + 1 bias + 3 pad)
BIG = 1.0e30


def build_program(T: int):
    nc = bacc.Bacc("TRN2", target_bir_lowering=False, debug=False, num_devices=NC)

    eg_t = nc.dram_tensor("eg_t", [V, 768], F32, kind="ExternalInput")
    wb_t = nc.dram_tensor("wb_t", [Vs, WBC], F32, kind="ExternalInput")
    whi_t = nc.dram_tensor("whi_t", [128, KC * NCH * CH], F16, kind="ExternalInput")
    whh_t = nc.dram_tensor("whh_t", [128, 8 * 768], F32, kind="ExternalInput")
    bhn_t = nc.dram_tensor("bhn_t", [B, 256], F32, kind="ExternalInput")
    bout_t = nc.dram_tensor("bout_t", [128, GROUPS * CH], F32, kind="ExternalInput")
    offs_t = nc.dram_tensor("offs_t", [128, 1], F32, kind="ExternalInput")
    voff_t = nc.dram_tensor("voff_t", [B, 1], F32, kind="ExternalInput")
    m1h_t = nc.dram_tensor("m1h_t", [64, B], F32, kind="ExternalInput")
    ht0_t = nc.dram_tensor("ht0_t", [128, 8 * 64], F32, kind="ExternalInput")
    hbm0_t = nc.dram_tensor("hbm0_t", [B, 256], F32, kind="ExternalInput")
    xg0_t = nc.dram_tensor("xg0_t", [B, 768], F32, kind="ExternalInput")
    logp_t = nc.dram_tensor("logp_t", [T * 128, GROUPS * CH], F32, kind="ExternalOutput")

    rg = [list(range(NC))]

    with tile.TileContext(nc) as tc:
        with (
            tc.tile_pool(name="const", bufs=1) as cpool,
            tc.tile_pool(name="gate", bufs=1) as gpool,
            tc.tile_pool(name="lg", bufs=2) as lpool,
            tc.tile_pool(name="stats", bufs=2) as tpool,
            tc.tile_pool(name="xgp", bufs=2) as xgpool,
            tc.tile_pool(name="wcp", bufs=1) as wcpool,
            tc.tile_pool(name="ps_rz", bufs=1, space="PSUM") as ps_rz_pool,
            tc.tile_pool(name="ps_hn", bufs=1, space="PSUM") as ps_hn_pool,
            tc.tile_pool(name="ps_proj", bufs=1, space="PSUM") as ps_proj_pool,
            tc.tile_pool(name="ps_tr", bufs=1, space="PSUM") as ps_tr_pool,
            tc.tile_pool(name="ps_wct", bufs=1, space="PSUM") as ps_wct_pool,
            tc.tile_pool(name="ps_ref", bufs=1, space="PSUM") as ps_ref_pool,
            tc.tile_pool(name="dram", bufs=2, space="DRAM") as dpool,
        ):
            # ---- resident loads ----
            ident = cpool.tile([128, 128], F32, name="ident")
            make_identity(nc, ident[:])
            id32 = ident[0:32, 0:32]
            id64 = ident[0:64, 0:64]
            whi = cpool.tile([128, KC * NCH * CH], F16, name="whi")
            nc.sync.dma_start(whi[:], whi_t.ap())
            whh = cpool.tile([128, 8 * 768], F32, name="whh")
            nc.sync.dma_start(whh[:], whh_t.ap())
            b_hn = cpool.tile([B, 256], F32, name="b_hn")
            nc.sync.dma_start(b_hn[:], bhn_t.ap())
            bout = cpool.tile([128, GROUPS * CH], F32, name="bout")
            nc.sync.dma_start(bout[:], bout_t.ap())
            offs = cpool.tile([128, 1], F32, name="offs")
            nc.sync.dma_start(offs[:], offs_t.ap())
            voff = cpool.tile([B, 1], F32, name="voff")
            nc.sync.dma_start(voff[:], voff_t.ap())
            m1h = cpool.tile([64, B], F32, name="m1h")
            nc.sync.dma_start(m1h[:], m1h_t.ap())
            bigt = cpool.tile([B, 8], F32, name="bigt")
            nc.vector.memset(bigt[:], BIG)

            # ping-pong state
            hT = [cpool.tile([128, 8 * 64], F32, name=f"hT{i}") for i in range(2)]
            hbm = [cpool.tile([B, 256], F32, name=f"hbm{i}") for i in range(2)]
            h16 = cpool.tile([128, 8 * 64], F16, name="h16")
            nc.sync.dma_start(hT[0][:], ht0_t.ap())
            nc.sync.dma_start(hbm[0][:], hbm0_t.ap())
            xg0 = xgpool.tile([B, 768], F32, name="xg", tag="xg")
            nc.sync.dma_start(xg0[:], xg0_t.ap())

            def emit_gh(t, rz_ps, hn_ps):
                """h-side GRU matmuls for step t (reads hT[t%2] = h(t-1))."""
                h = hT[t % 2]
                for c in range(8):
                    hf = h[:, c * 64 : c * 64 + 32]
                    hb = h[:, c * 64 + 32 : c * 64 + 64]
                    w = whh[:, c * 768 : (c + 1) * 768]
                    nc.tensor.matmul(rz_ps[:, 0:256], lhsT=hf, rhs=w[:, 0:256],
                                     start=(c == 0), stop=False)
                    nc.tensor.matmul(rz_ps[:, 256:512], lhsT=hb, rhs=w[:, 256:512],
                                     start=False, stop=(c == 7))
                    nc.tensor.matmul(hn_ps[:, 0:128], lhsT=hf, rhs=w[:, 512:640],
                                     start=(c == 0), stop=False)
                    nc.tensor.matmul(hn_ps[:, 128:256], lhsT=hb, rhs=w[:, 640:768],
                                     start=False, stop=(c == 7))

            def emit_fin(pend):
                # logp(tp) = logits - ln(s_glob): runs in the AG1 wait window
                tp, logits_p, s_glob_p = pend
                logz = tpool.tile([128, 1], F32, name="logz", tag="logz")
                nc.scalar.activation(logz[0:B, :], s_glob_p[:], AF.Ln)
                nc.vector.tensor_copy(logz[B : 2 * B, :], logz[0:B, :])
                nc.vector.tensor_copy(logz[2 * B :, :], logz[0 : 2 * B, :])
                bz = lpool.tile([128, GROUPS * CH], F32, name="bz", tag="bz")
                nc.vector.tensor_scalar(bz[:], logits_p[:], logz[:, 0:1], None,
                                        op0=ALU.subtract)
                nc.gpsimd.dma_start(logp_t.ap()[tp * 128 : (tp + 1) * 128, :], bz[:])

            pend = None
            # step-0 h-side prologue
            rz_ps_next = ps_rz_pool.tile([B, 512], F32, name="rz_ps", tag="rz")
            hn_ps_next = ps_hn_pool.tile([B, 256], F32, name="hn_ps", tag="hn")
            emit_gh(0, rz_ps_next, hn_ps_next)
            xg_next = xg0

            for t in range(T):
                rz_ps = rz_ps_next
                hn_ps = hn_ps_next
                xg = xg_next
                h_prev = hbm[t % 2]
                h_cur = hT[(t + 1) % 2]   # written by AG1(t)

                # ---- gates (batch-major; col order [f_r f_z b_r b_z]) ----
                # s_rz = sigmoid(rz_ps + Erz);  Erz has b_ih + b_hh folded in
                s_rz = gpool.tile([B, 512], F32, name="s_rz", tag="s_rz")
                nc.vector.tensor_add(s_rz[:], rz_ps[:], xg[:, 0:512])
                nc.scalar.activation(s_rz[:], s_rz[:], AF.Tanh, scale=0.5)
                nc.vector.tensor_scalar(s_rz[:], s_rz[:], 0.5, 0.5,
                                        op0=ALU.mult, op1=ALU.add)
                # n = tanh(i_n + r * (hn_ps + b_hn));  i_n = En (biases folded)
                h_n = gpool.tile([B, 256], F32, name="h_n", tag="h_n")
                nc.vector.tensor_add(h_n[:], hn_ps[:], b_hn[:])
                nc.vector.tensor_tensor(h_n[:, 0:128], s_rz[:, 0:128],
                                        h_n[:, 0:128], op=ALU.mult)
                nc.vector.tensor_tensor(h_n[:, 128:256], s_rz[:, 256:384],
                                        h_n[:, 128:256], op=ALU.mult)
                nc.vector.tensor_add(h_n[:], h_n[:], xg[:, 512:768])
                nc.scalar.activation(h_n[:], h_n[:], AF.Tanh)
                # d = (h_prev - n) * z ; h_new = n + d
                dz = gpool.tile([B, 256], F32, name="dz", tag="dz")
                nc.vector.tensor_sub(dz[:], h_prev[:], h_n[:])
                nc.vector.tensor_tensor(dz[:, 0:128], s_rz[:, 128:256],
                                        dz[:, 0:128], op=ALU.mult)
                nc.vector.tensor_tensor(dz[:, 128:256], s_rz[:, 384:512],
                                        dz[:, 128:256], op=ALU.mult)
                h_new = hbm[(t + 1) % 2]
                nc.vector.tensor_add(h_new[:], h_n[:], dz[:])

                # ---- transpose h_new, AllGather hidden ----
                tr_ps = ps_tr_pool.tile([128, 64], F32, name="tr_ps", tag="tr")
                nc.tensor.matmul(tr_ps[:, 0:32], lhsT=h_new[:, 0:128], rhs=id32,
                                 is_transpose=True, start=True, stop=False)
                nc.tensor.matmul(tr_ps[:, 32:64], lhsT=h_new[:, 128:256], rhs=id32,
                                 is_transpose=True, start=False, stop=True)
                ag1_sb = tpool.tile([128, 64], F32, name="ag1_sb", tag="ag1_sb")
                nc.vector.tensor_copy(ag1_sb[:], tr_ps[:, 0:64])
                ag1_in = dpool.tile([128, 64], F32, name="ag1_in", tag="ag1_in")
                nc.sync.dma_start(ag1_in[:], ag1_sb[:])
                ag1_out = dpool.tile([128 * NC, 64], F32, name="ag1_out",
                                     addr_space="Shared", tag="ag1_out")
                nc.gpsimd.collective_compute(
                    "AllGather", ALU.bypass, replica_groups=rg,
                    ins=[ag1_in.opt()], outs=[ag1_out.opt()])
                if pend is not None:
                    emit_fin(pend)
                    pend = None
                nc.sync.dma_start(
                    h_cur[:].rearrange("p (c q) -> p c q", c=8),
                    ag1_out[:].rearrange("(c p) q -> p c q", p=128))
                nc.gpsimd.dma_start(
                    h16[:].rearrange("p (c q) -> p c q", c=8),
                    ag1_out[:].rearrange("(c p) q -> p c q", p=128))

                def lh_of(c, hh):
                    if c < 8:
                        return hh[:, c * 64 : c * 64 + 32]
                    return hh[:, (c - 8) * 64 + 32 : (c - 8) * 64 + 64]

                # ---- output projection: single fp16 pass, resident weights ----
                pj = [ps_proj_pool.tile([128, 512], F32, name=f"pj{g}", tag=f"pj{g}")
                      for g in range(GROUPS)]
                for c in range(KC):
                    lh = lh_of(c, h16)
                    for ch in range(NCH):
                        g, j = divmod(ch, 4)
                        nc.tensor.matmul(
                            pj[g][32 * j : 32 * (j + 1), 0:CH], lhsT=lh,
                            rhs=whi[:, (c * NCH + ch) * CH : (c * NCH + ch + 1) * CH],
                            start=(c == 0), stop=(c == KC - 1),
                            skip_group_check=True, tile_position=(0, 32 * j))

                # ---- h-side matmuls for t+1 (PE, overlaps epilogue) ----
                if t + 1 < T:
                    rz_ps_next = ps_rz_pool.tile([B, 512], F32, name="rz_ps", tag="rz")
                    hn_ps_next = ps_hn_pool.tile([B, 256], F32, name="hn_ps", tag="hn")
                    emit_gh(t + 1, rz_ps_next, hn_ps_next)

                # ---- logits epilogue: bias, per-partition top-2 ----
                logits = lpool.tile([128, GROUPS * CH], F32, name="logits", tag="logits")
                for g in range(GROUPS):
                    nc.vector.tensor_add(logits[:, g * CH : (g + 1) * CH],
                                         pj[g][:, 0:CH], bout[:, g * CH : (g + 1) * CH])
                mx8 = tpool.tile([128, 8], F32, name="mx8", tag="mx8")
                ix8 = tpool.tile([128, 8], U32, name="ix8", tag="ix8")
                nc.vector.max(out=mx8[:], in_=logits[:])
                nc.vector.max_index(out=ix8[:], in_max=mx8[:], in_values=logits[:])
                # local vocab index = idx + 500*j + (idx >= 500) * 1500
                ixl = tpool.tile([128, 2], F32, name="ixl", tag="ixl")
                nc.vector.tensor_copy(ixl[:], ix8[:, 0:2])
                gm = tpool.tile([128, 2], F32, name="gm", tag="gm")
                nc.vector.tensor_scalar(gm[:], ixl[:], float(CH), 1500.0,
                                        op0=ALU.is_ge, op1=ALU.mult)
                nc.vector.tensor_add(ixl[:], ixl[:], gm[:])
                nc.vector.tensor_scalar(ixl[:], ixl[:], offs[:, 0:1], None, op0=ALU.add)

                # ---- per-b candidates: 4 partitions x top-2 -> [B, 8] ----
                cand8 = tpool.tile([B, 8], F32, name="cand8", tag="cand8")
                candi8 = tpool.tile([B, 8], F32, name="candi8", tag="candi8")
                for j in range(4):
                    nc.vector.tensor_copy(cand8[:, 2 * j : 2 * j + 2],
                                          mx8[32 * j : 32 * (j + 1), 0:2])
                    nc.vector.tensor_copy(candi8[:, 2 * j : 2 * j + 2],
                                          ixl[32 * j : 32 * (j + 1), 0:2])
                # fp16 top-2 of the 8 (unordered {i_a, i_b}): threshold at v2
                v8d = tpool.tile([B, 8], F32, name="v8d", tag="v8d")
                nc.vector.max(out=v8d[:], in_=cand8[:])
                v2 = v8d[:, 1:2]
                msk = tpool.tile([B, 8], U32, name="msk", tag="msk")
                nc.vector.tensor_scalar(msk[:], cand8[:], v2, None, op0=ALU.is_ge)
                isel = tpool.tile([B, 8], F32, name="isel", tag="isel")
                nc.vector.tensor_copy(isel[:], bigt[:])
                nc.vector.copy_predicated(isel[:], msk[:], candi8[:])
                i1 = tpool.tile([B, 1], F32, name="i1", tag="i1")
                nc.vector.tensor_reduce(i1[:], isel[:], axis=AX.X, op=ALU.min)
                msk2 = tpool.tile([B, 8], U32, name="msk2", tag="msk2")
                nc.vector.tensor_scalar(msk2[:], candi8[:], i1[:, 0:1], None,
                                        op0=ALU.is_equal)
                nc.vector.copy_predicated(isel[:], msk2[:], bigt[:])
                i2 = tpool.tile([B, 1], F32, name="i2", tag="i2")
                nc.vector.tensor_reduce(i2[:], isel[:], axis=AX.X, op=ALU.min)

                # ---- refinement: exact fp32 logits for the 2 candidates ----
                ci64 = tpool.tile([64, 1], U32, name="ci64", tag="ci64")
                nc.vector.tensor_copy(ci64[0:B, :], i1[:])
                nc.vector.tensor_copy(ci64[B : 2 * B, :], i2[:])
                wc = wcpool.tile([64, WBC], F32, name="wc", tag="wc")
                nc.gpsimd.indirect_dma_start(
                    out=wc[:], out_offset=None, in_=wb_t.ap(),
                    in_offset=bass.IndirectOffsetOnAxis(ap=ci64[:, 0:1], axis=0))
                wct_ps = [ps_wct_pool.tile([128, 512], F32, name=f"wct{i}", tag=f"wct{i}")
                          for i in range(2)]
                for c in range(KC):
                    q, r = divmod(c, 8)
                    nc.tensor.matmul(wct_ps[q][:, r * 64 : (r + 1) * 64],
                                     lhsT=wc[:, c * 128 : (c + 1) * 128], rhs=id64,
                                     is_transpose=True, start=(r == 0), stop=(r == 7))
                wcT = wcpool.tile([128, KC * 64], F32, name="wcT", tag="wcT")
                nc.vector.tensor_copy(wcT[:, 0:512], wct_ps[0][:])
                nc.vector.tensor_copy(wcT[:, 512:1024], wct_ps[1][:])
                ref_ps = ps_ref_pool.tile([64, B], F32, name="ref_ps", tag="ref")
                for c in range(KC):
                    nc.tensor.matmul(ref_ps[:], lhsT=wcT[:, c * 64 : (c + 1) * 64],
                                     rhs=lh_of(c, hT[(t + 1) % 2]),
                                     start=(c == 0), stop=(c == KC - 1))
                # refined[p, b] valid at b == p%32; add bias, select diagonal
                refined = tpool.tile([64, B], F32, name="refined", tag="refined")
                nc.vector.tensor_scalar(refined[:], ref_ps[:], wc[:, 2048:2049], None,
                                        op0=ALU.add)
                nc.vector.tensor_tensor(refined[:], refined[:], m1h[:], op=ALU.mult)
                rv = tpool.tile([64, 1], F32, name="rv", tag="rv")
                nc.vector.reduce_sum(rv[:], refined[:], axis=AX.X)
                # exact local max + its index
                v1r = rv[0:B, :]
                v2rc = tpool.tile([B, 1], F32, name="v2rc", tag="v2rc")
                nc.vector.tensor_copy(v2rc[:], rv[B : 2 * B, :])
                ag2_sb = tpool.tile([B, 3], F32, name="ag2_sb", tag="ag2_sb")
                m_ex = ag2_sb[:, 0:1]
                nc.vector.tensor_tensor(m_ex, v1r, v2rc[:], op=ALU.max)
                ge = tpool.tile([B, 1], U32, name="ge", tag="ge")
                nc.vector.tensor_tensor(ge[:], v1r, v2rc[:], op=ALU.is_ge)
                i_ex = tpool.tile([B, 1], F32, name="i_ex", tag="i_ex")
                nc.vector.tensor_copy(i_ex[:], i2[:])
                nc.vector.copy_predicated(i_ex[:], ge[:], i1[:])
                nc.vector.tensor_scalar(ag2_sb[:, 1:2], i_ex[:], voff[:, 0:1], None,
                                        op0=ALU.add)

                # ---- local sum-exp (no max-sub: |logits| is small, no overflow) ----
                sparts = tpool.tile([128, 2], F32, name="sparts", tag="sparts")
                esc = tpool.tile([128, CH], F32, name="esc", tag="esc", bufs=1)
                for g in range(GROUPS):
                    nc.scalar.activation(esc[:], logits[:, g * CH : (g + 1) * CH],
                                         AF.Exp,
                                         accum_out=sparts[:, g : g + 1])
                tand = tpool.tile([B, 1], F32, name="tand", tag="tand")
                nc.scalar.activation(tand[:], bigt[:, 0:1], AF.Tanh)
                s128 = tpool.tile([128, 1], F32, name="s128", tag="s128")
                nc.vector.tensor_add(s128[:], sparts[:, 0:1], sparts[:, 1:2])
                scand = tpool.tile([B, 4], F32, name="scand", tag="scand")
                for j in range(4):
                    nc.vector.tensor_copy(scand[:, j : j + 1],
                                          s128[32 * j : 32 * (j + 1), :])
                nc.vector.reduce_sum(ag2_sb[:, 2:3], scand[:], axis=AX.X)

                # ---- AG2: (m_ex, i_ex, s_loc) ----
                ag2_in = dpool.tile([B, 3], F32, name="ag2_in", tag="ag2_in")
                nc.sync.dma_start(ag2_in[:], ag2_sb[:])
                ag2_out = dpool.tile([B * NC, 3], F32, name="ag2_out",
                                     addr_space="Shared", tag="ag2_out")
                nc.gpsimd.collective_compute(
                    "AllGather", ALU.bypass, replica_groups=rg,
                    ins=[ag2_in.opt()], outs=[ag2_out.opt()])
                unp2 = tpool.tile([B, 24], F32, name="unp2", tag="unp2")
                nc.sync.dma_start(
                    unp2[:].rearrange("b (r c) -> b r c", r=NC),
                    ag2_out[:].rearrange("(r b) c -> b r c", b=B))
                vals = bass.AP(unp2.tensor, unp2[:].offset,
                               [unp2[:].ap[0], [3, 8]])
                idxs = bass.AP(unp2.tensor, unp2[:].offset + 1,
                               [unp2[:].ap[0], [3, 8]])
                svals = bass.AP(unp2.tensor, unp2[:].offset + 2,
                                [unp2[:].ap[0], [3, 8]])
                m_glob = tpool.tile([B, 1], F32, name="m_glob", tag="m_glob")
                nc.vector.reduce_max(m_glob[:], vals, axis=AX.X)
                gmsk = tpool.tile([B, 8], U32, name="gmsk", tag="gmsk")
                nc.vector.tensor_scalar(gmsk[:], vals, m_glob[:], None, op0=ALU.is_equal)
                gisel = tpool.tile([B, 8], F32, name="gisel", tag="gisel")
                nc.vector.tensor_copy(gisel[:], bigt[:])
                nc.vector.copy_predicated(gisel[:], gmsk[:], idxs)
                i_glob = tpool.tile([B, 1], F32, name="i_glob", tag="i_glob")
                nc.vector.tensor_reduce(i_glob[:], gisel[:], axis=AX.X, op=ALU.min)
                # logZ = ln(sum_k s_k)  (Ln/bz/logp-write deferred to next step)
                s_glob = tpool.tile([B, 1], F32, name="s_glob", tag="s_glob")
                nc.vector.reduce_sum(s_glob[:], svals, axis=AX.X)
                pend = (t, logits, s_glob)

                # ---- E' gather for t+1 ----
                if t + 1 < T:
                    tok = tpool.tile([B, 1], U32, name="tok", tag="tok")
                    nc.vector.tensor_copy(tok[:], i_glob[:])
                    xg_next = xgpool.tile([B, 768], F32, name="xg", tag="xg")
                    nc.gpsimd.indirect_dma_start(
                        out=xg_next[:], out_offset=None, in_=eg_t.ap(),
                        in_offset=bass.IndirectOffsetOnAxis(ap=tok[:, 0:1], axis=0))

            if pend is not None:
                emit_fin(pend)
                pend = None


    nc.compile()
    return nc


def prep_inputs(inputs, hidden, emb, w_ih_f, w_hh_f, b_ih_f, b_hh_f,
                w_ih_b, w_hh_b, b_ih_b, b_hh_b, w_out, b_out):
    """Build the per-core input maps (all numpy, host-side sharding)."""
    emb = np.ascontiguousarray(np.asarray(emb), dtype=np.float32)
    w_out = np.ascontiguousarray(np.asarray(w_out), dtype=np.float32)
    b_out = np.asarray(b_out).astype(np.float32)
    tok0 = np.asarray(inputs)[:, 0].astype(np.int64)
    hidden = np.asarray(hidden)
    h_f0, h_b0 = hidden[0], hidden[1]                           # (B, H)

    ht0 = np.empty((128, 8, 64), dtype=np.float32)
    ht0[:, :, 0:32] = np.ascontiguousarray(h_f0.T).reshape(8, 128, B).transpose(1, 0, 2)
    ht0[:, :, 32:64] = np.ascontiguousarray(h_b0.T).reshape(8, 128, B).transpose(1, 0, 2)
    ht0 = ht0.reshape(128, 8 * 64)

    wihf, whhf = np.asarray(w_ih_f), np.asarray(w_hh_f)
    wihb, whhb = np.asarray(w_ih_b), np.asarray(w_hh_b)
    bihf, bhhf = np.asarray(b_ih_f), np.asarray(b_hh_f)
    bihb, bhhb = np.asarray(b_ih_b), np.asarray(b_hh_b)

    # E' = emb @ w_ih.T (+ biases folded) for both directions, full (V, 3H)
    Ef = emb @ wihf.T.astype(np.float32)                        # (V, 3H)
    Eb = emb @ wihb.T.astype(np.float32)

    m1h = np.zeros((64, B), dtype=np.float32)
    for p in range(64):
        m1h[p, p % B] = 1.0

    of = np.empty((128, 1), dtype=np.float32)
    for j in range(4):
        of[32 * j : 32 * (j + 1), 0] = j * CH

    in_maps = []
    for k in range(NC):
        v0 = Vs * k
        sl = [slice(g * H + Hs * k, g * H + Hs * (k + 1)) for g in range(3)]

        # whi: [p, (c*8+ch)*500+f] = w_out[v0+ch*500+f, c*128+p]
        w_oT = np.ascontiguousarray(w_out[v0 : v0 + Vs, :].T)   # (2048, 4000)
        whi = w_oT.reshape(KC, 128, NCH, CH).transpose(1, 0, 2, 3) \
            .reshape(128, KC * NCH * CH).astype(np.float16).copy()

        wb = np.zeros((Vs, WBC), dtype=np.float32)
        wb[:, 0:2048] = w_out[v0 : v0 + Vs, :]
        wb[:, 2048] = b_out[v0 : v0 + Vs]

        def gcat(wf, wb_):
            cols = [wf[sl[0]].T, wf[sl[1]].T, wb_[sl[0]].T, wb_[sl[1]].T,
                    wf[sl[2]].T, wb_[sl[2]].T]
            cat = np.concatenate(cols, axis=1)                   # (1024, 768)
            return cat.reshape(8, 128, 768).transpose(1, 0, 2) \
                .reshape(128, 8 * 768).astype(np.float32).copy()

        # E' slice: [f_r f_z b_r b_z f_n b_n] with biases folded
        eg = np.empty((V, 768), dtype=np.float32)
        eg[:, 0:128] = Ef[:, sl[0]] + (bihf[sl[0]] + bhhf[sl[0]])
        eg[:, 128:256] = Ef[:, sl[1]] + (bihf[sl[1]] + bhhf[sl[1]])
        eg[:, 256:384] = Eb[:, sl[0]] + (bihb[sl[0]] + bhhb[sl[0]])
        eg[:, 384:512] = Eb[:, sl[1]] + (bihb[sl[1]] + bhhb[sl[1]])
        eg[:, 512:640] = Ef[:, sl[2]] + bihf[sl[2]]
        eg[:, 640:768] = Eb[:, sl[2]] + bihb[sl[2]]

        def bcast(v):
            return np.broadcast_to(v.astype(np.float32), (B, v.size)).copy()

        b_hn_ = bcast(np.concatenate([bhhf[sl[2]], bhhb[sl[2]]]))

        bo = b_out[v0 : v0 + Vs].reshape(GROUPS, 4, CH)
        boutt = np.empty((128, GROUPS * CH), dtype=np.float32)
        for g in range(GROUPS):
            for j in range(4):
                boutt[32 * j : 32 * (j + 1), g * CH : (g + 1) * CH] = bo[g, j]

        hbm0 = np.concatenate([h_f0[:, Hs * k : Hs * (k + 1)],
                               h_b0[:, Hs * k : Hs * (k + 1)]], axis=1) \
            .astype(np.float32).copy()

        voff = np.full((B, 1), float(v0), dtype=np.float32)
        xg0 = np.ascontiguousarray(eg[tok0])                     # (B, 768)

        in_maps.append({
            "eg_t": eg, "wb_t": wb, "whi_t": whi,
            "whh_t": gcat(whhf, whhb), "bhn_t": b_hn_,
            "bout_t": boutt, "offs_t": of, "voff_t": voff, "m1h_t": m1h,
            "ht0_t": ht0, "hbm0_t": hbm0, "xg0_t": xg0,
        })
    return in_maps


_CACHE = {}


def _get_program(T, **kw):
    key = (T, tuple(sorted(kw.items())))
    if key not in _CACHE:
        _CACHE[key] = build_program(T, **kw)
    return _CACHE[key]


def run(T, in_maps, trace=False):
    nc = _get_program(T)
    res = bass_utils.run_bass_kernel_spmd(
        nc, in_maps, core_ids=list(range(NC)), trace=trace)
    outs = []
    for k in range(NC):
        arr = res.results[k]["logp_t"].reshape(T, 4, B, GROUPS, CH)
        outs.append(arr.transpose(2, 0, 3, 1, 4).reshape(B, T, Vs))
    return np.concatenate(outs, axis=2), res


def kernel(inputs, hidden, emb, w_ih_f, w_hh_f, b_ih_f, b_hh_f,
           w_ih_b, w_hh_b, b_ih_b, b_hh_b, w_out, b_out, output_len):
    T = int(output_len)
    in_maps = prep_inputs(inputs, hidden, emb, w_ih_f, w_hh_f, b_ih_f, b_hh_f,
                          w_ih_b, w_hh_b, b_ih_b, b_hh_b, w_out, b_out)
    out, _ = run(T, in_maps)
    return out
